# revision 32
# baseline (speedup 1.0000x reference)
"""Bass/Trainium2 kernel for nn_BellmanLoss (8-core data-parallel).

Math: the reference's scatter makes Q_new differ from Q0 only at
a_i = argmax_j(actions[i, j]) (first max), so

    loss = sum_i (Q0[i, a_i] - target_i)^2
    target_i = r_i + 0.9 * max_a Qn[i, a] * (1 - done_i),  done_i = (states1[i,0] == 666)

Per core: 8192 rows, CH=512 batch columns per tick, 32 ticks (even=state0,
odd=state1 chunks). MLP runs feature-major (h^T = [features, batch]):
  mm1: fp8 non-DR (K=128), N=512  -> h1p PSUM [128,2,512]
  relu1: ACT/DVE copy PSUM->SBUF fp8 (+b1)
  mm2: fp8 DoubleRow (K=256 packed), N=512 -> h2p PSUM [128,512] per m
  relu2: per-m copies -> h2s fp8
  mm3: fp8 DoubleRow, Q^T [18,512] packed 4x32-partition groups per qt bank
  stack: PSUM->SBUF bf16 (+b3)
  dma_start_transpose: qs [128,512] -> qbuf [128,4,128] batch-major
Epilogue (batch-major, small FD): argmax-onehot select of Q0, max of Qn,
target, per-partition loss partials. Host does layout-only prep and the
final 1024-element sum.
"""

import os
os.environ.setdefault("NEURON_RT_ENABLE_DGE_NOTIFICATIONS", "1")
import numpy as np
import ml_dtypes

import concourse.bass as bass
import concourse.mybir as mybir
import concourse.tile as tile
from concourse import bacc
from concourse.bass_utils import run_bass_kernel_spmd

# Problem constants (hardcoded per contract)
B, S, H, A = 65536, 128, 256, 18
NCORES = 8
BC = B // NCORES          # 8192 rows per core
CH = 512                  # batch columns per tick
T = 2 * (BC // CH)        # 32 ticks (x0/x1 interleaved)
NQ = BC // CH // 2        # 8 qt tiles (each: 2 chunk-pairs x (Q0,Qn))
GR = BC // 128            # 64 batch blocks of 128 rows
LOADCOLS = 1024           # x DMA tile columns
DONE = 666.0
DISC = 0.9

FP8 = mybir.dt.float8e4
BF16 = mybir.dt.bfloat16
FP16 = mybir.dt.float16
F32 = mybir.dt.float32
I8 = mybir.dt.int8
AF = mybir.ActivationFunctionType
OP = mybir.AluOpType
AX = mybir.AxisListType
DR = mybir.MatmulPerfMode.DoubleRow

NP_FP8 = ml_dtypes.float8_e4m3
NP_BF16 = ml_dtypes.bfloat16

USE_DR = os.environ.get("BELLMAN_DR", "1") == "1"
USE_DMAT = os.environ.get("BELLMAN_DMAT", "1") == "1"


def _build_program():
    nc = bacc.Bacc("TRN2", target_bir_lowering=False, debug=False)

    x0t = nc.dram_tensor("x0t", [128, BC], FP8, kind="ExternalInput").ap()
    x1t = nc.dram_tensor("x1t", [128, BC], FP8, kind="ExternalInput").ap()
    actb = nc.dram_tensor("actb", [128, GR * A], I8, kind="ExternalInput").ap()
    rewb = nc.dram_tensor("rewb", [128, GR], F32, kind="ExternalInput").ap()
    s1b = nc.dram_tensor("s1b", [128, GR], F32, kind="ExternalInput").ap()
    w1 = nc.dram_tensor("w1", [S, H], FP8, kind="ExternalInput").ap()
    w2km = nc.dram_tensor("w2km", [128, 2 * H], FP8, kind="ExternalInput").ap()
    w3s = nc.dram_tensor("w3s", [128, 2 * 32], FP8, kind="ExternalInput").ap()
    b1d = nc.dram_tensor("b1d", [128, 2], F32, kind="ExternalInput").ap()
    b2d = nc.dram_tensor("b2d", [128, 2], F32, kind="ExternalInput").ap()
    b3st = nc.dram_tensor("b3st", [128, 1], F32, kind="ExternalInput").ap()
    iotad = nc.dram_tensor("iotad", [128, A], FP16, kind="ExternalInput").ap()
    outp = nc.dram_tensor("outp", [128, 1], F32, kind="ExternalOutput").ap()

    from contextlib import ExitStack

    with tile.TileContext(nc) as tc, ExitStack() as ctx:
        singles = ctx.enter_context(tc.tile_pool(name="singles", bufs=1))
        xpool = ctx.enter_context(tc.tile_pool(name="xpool", bufs=3))
        h1spool = ctx.enter_context(tc.tile_pool(name="h1s", bufs=4))
        h2spool = ctx.enter_context(tc.tile_pool(name="h2s", bufs=6))
        big = ctx.enter_context(tc.tile_pool(name="big", bufs=1))
        ps_h1 = ctx.enter_context(tc.tile_pool(name="ps_h1", bufs=2, space="PSUM"))
        ps_h2 = ctx.enter_context(tc.tile_pool(name="ps_h2", bufs=1, space="PSUM"))
        ps_qt = ctx.enter_context(tc.tile_pool(name="ps_qt", bufs=1, space="PSUM"))

        # --- constants / per-core staging loads (scalar queue, early) ---
        w1_s = singles.tile([S, H], FP8)
        nc.scalar.dma_start(out=w1_s, in_=w1)
        w2_s = singles.tile([128, 2, H], FP8, tag="w2")
        nc.scalar.dma_start(
            out=w2_s[:, :, :].rearrange("p a b -> p (a b)"), in_=w2km)
        w3_s = singles.tile([128, 2, 32], FP8, tag="w3")
        nc.scalar.dma_start(
            out=w3_s[:, :, :].rearrange("p a b -> p (a b)"), in_=w3s)
        b1_s = singles.tile([128, 2], F32, tag="b1")
        nc.scalar.dma_start(out=b1_s, in_=b1d)
        b2_s = singles.tile([128, 2], F32, tag="b2")
        nc.scalar.dma_start(out=b2_s, in_=b2d)
        b3_s = singles.tile([128, 1], F32, tag="b3")
        nc.scalar.dma_start(out=b3_s, in_=b3st)
        iota_s = singles.tile([128, A], FP16, tag="iota")
        nc.scalar.dma_start(out=iota_s, in_=iotad)
        actb_s = singles.tile([128, GR * A], I8, tag="actb")
        rewb_s = singles.tile([128, GR], F32, tag="rewb")
        s1b_s = singles.tile([128, GR], F32, tag="s1b")

        # qs: stacked Q^T in SBUF bf16 (stack copies write, dma-transpose reads)
        qs = big.tile([128, NQ, CH], BF16, tag="qs")
        # qbuf: batch-major Q (partition = batch-within-128-block)
        qbuf = big.tile([128, NQ, 4, 128], BF16, tag="qbuf")

        # epilogue tiles
        actf = big.tile([128, GR * A], FP16, tag="actf")
        score = big.tile([128, GR, A], FP16, tag="score")
        rowmax = big.tile([128, GR], FP16, tag="rowmax")
        onehot = big.tile([128, GR, A], BF16, tag="onehot")
        donem = big.tile([128, GR], F32, tag="donem")
        fac = big.tile([128, GR], F32, tag="fac")
        prod = big.tile([128, GR, A], BF16, tag="prod")
        q0sel = big.tile([128, GR], F32, tag="q0sel")
        maxqn = big.tile([128, GR], F32, tag="maxqn")
        t1 = big.tile([128, GR], F32, tag="t1")
        t2 = big.tile([128, GR], F32, tag="t2")
        diff = big.tile([128, GR], F32, tag="diff")
        sq = big.tile([128, GR], F32, tag="sq")
        acc = big.tile([128, 1], F32, tag="acc")

        # h2 PSUM: one 3-bank region, bank rotation (2t)%3/(2t+1)%3 per tick;
        # the pair is always expressible as a (possibly reversed) slice, so
        # relu2 evacuates both m-halves in ONE op.
        h2region = ps_h2.tile([128, 3, CH], F32, tag="h2r")

        xL = {}
        h1p_t, h1s_t, h2s_t, qt_q = {}, {}, {}, {}

        # greedy engine balancer for PSUM-evacuation copies
        eng_load = {"a": 0.0, "v": 0.0}

        def pick_engine(act_cost, dve_cost):
            if eng_load["a"] + act_cost <= eng_load["v"] + dve_cost:
                eng_load["a"] += act_cost
                return nc.scalar
            eng_load["v"] += dve_cost
            return nc.vector

        def do_dma(li):
            x0L = xpool.tile([128, LOADCOLS], FP8, tag="x0")
            x1L = xpool.tile([128, LOADCOLS], FP8, tag="x1")
            nc.sync.dma_start(out=x0L,
                              in_=x0t[:, li * LOADCOLS:(li + 1) * LOADCOLS])
            nc.sync.dma_start(out=x1L,
                              in_=x1t[:, li * LOADCOLS:(li + 1) * LOADCOLS])
            xL[li] = (x0L, x1L)

        # small first loads so tick 0/1 start ~2us earlier
        x0m = singles.tile([128, CH], FP8, tag="x0m")
        nc.sync.dma_start(out=x0m, in_=x0t[:, 0:CH])
        x1m = singles.tile([128, CH], FP8, tag="x1m")
        nc.sync.dma_start(out=x1m, in_=x1t[:, 0:CH])

        def xs_for(t):
            if t == 0:
                return x0m[:, :]
            if t == 1:
                return x1m[:, :]
            c, pa = t // 2, t % 2
            li = (c * CH) // LOADCOLS
            ci = (c * CH) % LOADCOLS // CH
            return xL[li][pa][:, ci * CH:(ci + 1) * CH]

        def st_mm1(t):
            h1p = ps_h1.tile([128, 2, CH], F32, tag="h1p", name=f"h1p_{t}")
            xs = xs_for(t)
            for m in range(2):
                nc.tensor.matmul(h1p[:, m, :], w1_s[:, m * 128:(m + 1) * 128],
                                 xs, start=True, stop=True)
            h1p_t[t] = h1p

        def emit_relu(eng, dst, src, bias_ap):
            if eng is nc.scalar:
                nc.scalar.activation(dst, src, AF.Relu, bias=bias_ap, scale=1.0)
            else:
                nc.vector.tensor_scalar(dst, src, bias_ap, 0.0, OP.add, OP.max)

        def st_relu1(t):
            h1s = h1spool.tile([128, 2, CH], FP8, tag="h1s", name=f"h1s_{t}")
            emit_relu(pick_engine(1114, 1282),
                      h1s[:, :, :].rearrange("p a b -> p (a b)"),
                      h1p_t.pop(t)[:, :, :].rearrange("p a b -> p (a b)"),
                      b1_s[:, 0:1])
            h1s_t[t] = h1s

        def h2slice(t):
            a = (2 * t) % 3
            if a == 0:
                return h2region[:, 0:2, :]
            if a == 1:
                return h2region[:, 1:3, :]
            return h2region[:, 2::-2, :]   # banks (2, 0)

        def st_mm2(t):
            h1s = h1s_t.pop(t)
            sl = h2slice(t)
            for m in range(2):
                if USE_DR:
                    nc.tensor.matmul(sl[:, m, :],
                                     w2_s[:, :, m * 128:(m + 1) * 128],
                                     h1s[:, :, :], start=True, stop=True,
                                     perf_mode=DR)
                else:
                    for k in range(2):
                        nc.tensor.matmul(sl[:, m, :],
                                         w2_s[:, k, m * 128:(m + 1) * 128],
                                         h1s[:, k, :],
                                         start=(k == 0), stop=(k == 1))

        def st_relu2(t):
            h2s = h2spool.tile([128, 2, CH], FP8, tag="h2s", name=f"h2s_{t}")
            emit_relu(pick_engine(1114, 1282), h2s[:, :, :], h2slice(t),
                      b2_s[:, 0:1])
            h2s_t[t] = h2s

        def st_mm3(c):
            # chunk-pair c: Q0 from h2s[2c] (state0), Qn from h2s[2c+1];
            # col-tiled pairs run concurrently on different PE column groups
            q = c // 2
            gp = (c % 2) * 2
            if c % 2 == 0:
                qt_q[q] = ps_qt.tile([128, CH], F32, tag="qt", name=f"qt_{q}")
            qt = qt_q[q]
            h2s0 = h2s_t.pop(2 * c)
            h2s1 = h2s_t.pop(2 * c + 1)
            for k in range(2):
                for gi, h2sx in ((gp, h2s0), (gp + 1, h2s1)):
                    po = gi * 32
                    nc.tensor.matmul(qt[po:po + A, :], w3_s[:, k, 0:A],
                                     h2sx[:, k, :], start=(k == 0),
                                     stop=(k == 1), tile_position=(0, po))

        def st_stack(q):
            # PSUM f32 -> SBUF bf16 with b3 bias (per stacked partition)
            eng = pick_engine(690, 750)
            if eng is nc.scalar:
                nc.scalar.activation(qs[:, q, :], qt_q[q], AF.Identity,
                                     bias=b3_s[:, 0:1], scale=1.0)
            else:
                nc.vector.tensor_scalar(qs[:, q, :], qt_q[q], b3_s[:, 0:1],
                                        None, OP.add)
            qt_q.pop(q)

        def st_dmaT(q):
            nc.sync.dma_start_transpose(out=qbuf[:, q, :, :], in_=qs[:, q, :])

        def ep_front(qq):
            # quarter qq of argmax/onehot of actions; done mask; factor
            gsl = slice(qq * 16, (qq + 1) * 16)
            asl = slice(qq * 16 * A, (qq + 1) * 16 * A)
            a3 = actf[:, asl].rearrange("p (g a) -> p g a", a=A)
            iot_b = iota_s[:, None, :].broadcast_to([128, 16, A])
            nc.scalar.activation(actf[:, asl], actb_s[:, asl], AF.Copy,
                                 scale=32.0)
            eng_load["a"] += 530
            nc.vector.tensor_tensor(score[:, gsl, :], a3, iot_b, OP.subtract)
            nc.vector.tensor_reduce(rowmax[:, gsl], score[:, gsl, :], AX.X,
                                    OP.max)
            nc.vector.tensor_tensor(
                onehot[:, gsl, :], score[:, gsl, :],
                rowmax[:, gsl, None].broadcast_to([128, 16, A]), OP.is_equal)
            eng_load["v"] += 1100
            if qq == 0:
                nc.vector.tensor_scalar(donem, s1b_s, DONE, None, OP.is_equal)
                nc.vector.tensor_scalar(fac, donem, -DISC, DISC, OP.mult,
                                        OP.add)
                eng_load["v"] += 370

        def ep_tail(half):
            # half = 0..3: q pair (2*half, 2*half+1), blocks 16*half..16*half+15
            q0_ = half * 2
            qh = qbuf[:, q0_:q0_ + 2, :, :]          # [128, 2, 4, 128]
            gsl = slice(half * 16, (half + 1) * 16)
            oh3 = onehot[:, :, :]                     # [128, GR, A]
            for pair in range(2):
                # blocks g' = 8q + 2w + pair; qbuf group = 2*pair (+1 Qn)
                q0ap = qh[:, :, :, 64 * pair:64 * pair + A]       # [128,2,4,A]
                qnap = qh[:, :, :, 64 * pair + 32:64 * pair + 32 + A]
                ohap = oh3[:, half * 16 + pair:(half + 1) * 16:2, :] \
                    .rearrange("p (q w) a -> p q w a", q=2)
                prap = prod[:, half * 16 + pair:(half + 1) * 16:2, :] \
                    .rearrange("p (q w) a -> p q w a", q=2)
                nc.vector.tensor_tensor(prap, ohap, q0ap, OP.mult)
                nc.vector.tensor_reduce(
                    q0sel[:, half * 16 + pair:(half + 1) * 16:2]
                    .rearrange("p (q w) -> p q w", q=2), prap, AX.X, OP.add)
                nc.vector.tensor_reduce(
                    maxqn[:, half * 16 + pair:(half + 1) * 16:2]
                    .rearrange("p (q w) -> p q w", q=2), qnap, AX.X, OP.max)
            nc.vector.tensor_tensor(t1[:, gsl], maxqn[:, gsl], fac[:, gsl],
                                    OP.mult)
            nc.vector.tensor_tensor(t2[:, gsl], t1[:, gsl], rewb_s[:, gsl],
                                    OP.add)
            nc.vector.tensor_tensor(diff[:, gsl], q0sel[:, gsl], t2[:, gsl],
                                    OP.subtract)
            nc.vector.tensor_tensor(sq[:, gsl], diff[:, gsl], diff[:, gsl],
                                    OP.mult)
            eng_load["v"] += 2400

        # ---- main software-pipelined loop ----
        # Prime the PE HAM clock-gate with ~3.5us of throwaway matmuls on
        # uninitialized SBUF (no DMA deps, so they start right after the
        # preamble) so the array is at 2.4GHz when the real work lands.
        prime_ps = ps_qt.tile([128, CH], F32, tag="qt", name="prime")
        for _ in range(9):
            nc.tensor.matmul(prime_ps, qs[:, 0, 0:128], qs[:, 0, :],
                             start=True, stop=True)
        do_dma(0)
        do_dma(1)
        PASS_PER_LOAD = 2 * LOADCOLS // CH   # ticks covered per load pair
        for t in range(T + 10):
            nt = t + 2 * PASS_PER_LOAD
            if nt < T and nt % PASS_PER_LOAD == 0:
                do_dma(nt // PASS_PER_LOAD)
            if t == 3:
                nc.sync.dma_start(out=actb_s, in_=actb)
                nc.sync.dma_start(out=rewb_s, in_=rewb)
                nc.sync.dma_start(out=s1b_s, in_=s1b)
            if t in (7, 9, 11, 13):
                ep_front((t - 7) // 2)
            if t < T:
                st_mm1(t)
            if 0 <= t - 2 < T:
                st_mm2(t - 2)
            if t >= 5 and (t - 5) % 2 == 0 and (t - 5) // 2 < T // 2:
                st_mm3((t - 5) // 2)
            if 0 <= t - 3 < T:
                st_relu2(t - 3)
            if 0 <= t - 1 < T:
                st_relu1(t - 1)
            if t >= 8 and (t - 8) % 4 == 0:
                q = (t - 8) // 4
                if q < NQ:
                    st_stack(q)
            if t >= 9 and (t - 9) % 4 == 0:
                q = (t - 9) // 4
                if q < NQ:
                    st_dmaT(q)
            if t >= 15 and (t - 15) % 8 == 0 and (t - 15) // 8 < 4:
                ep_tail((t - 15) // 8)
        nc.vector.tensor_reduce(acc, sq, AX.X, OP.add)
        nc.scalar.dma_start(out=outp, in_=acc)

    nc.compile()
    return nc


_CACHE = {}


def _get_program():
    if "nc" not in _CACHE:
        _CACHE["nc"] = _build_program()
    return _CACHE["nc"]


def _block_perm():
    # qbuf block order g' -> source batch block b
    perm = np.empty(GR, np.int64)
    for gp in range(GR):
        q, r = divmod(gp, 8)
        w, pair = divmod(r, 2)
        perm[gp] = (2 * q + pair) * 4 + w
    return perm


def _prep_in_maps(inputs):
    st0 = np.asarray(inputs["states0"], dtype=np.float32)
    st1 = np.asarray(inputs["states1"], dtype=np.float32)
    act = np.asarray(inputs["actions"], dtype=np.int32)
    rew = np.asarray(inputs["rewards"], dtype=np.float32)
    W1 = np.asarray(inputs["W1"], dtype=np.float32)
    W2 = np.asarray(inputs["W2"], dtype=np.float32)
    W3 = np.asarray(inputs["W3"], dtype=np.float32)
    b1 = np.asarray(inputs["b1"], dtype=np.float32)
    b2 = np.asarray(inputs["b2"], dtype=np.float32)
    b3 = np.asarray(inputs["b3"], dtype=np.float32)

    # sanitize DONE sentinel (666 > fp8e4m3 max); done rows' Qn is masked out
    s1col = st1[:, 0].copy()
    st1f = st1.copy()
    st1f[:, 0] = np.where(s1col == DONE, 0.0, s1col)

    w1f = W1.astype(NP_FP8)
    w2km = np.ascontiguousarray(
        W2.reshape(2, 128, H).transpose(1, 0, 2)).astype(NP_FP8).reshape(128, 2 * H)
    w3p = np.zeros((128, 2, 32), np.float32)
    w3p[:, :, :A] = W3.reshape(2, 128, A).transpose(1, 0, 2)
    w3s = w3p.astype(NP_FP8).reshape(128, 2 * 32)
    b1m = np.ascontiguousarray(b1.reshape(2, 128).T)
    b2m = np.ascontiguousarray(b2.reshape(2, 128).T)
    b3stk = np.zeros((128, 1), np.float32)
    for g in range(4):
        b3stk[g * 32:g * 32 + A, 0] = b3
    iota = np.ascontiguousarray(
        np.broadcast_to(np.arange(A, dtype=np.float16), (128, A)))

    perm = _block_perm()
    act8 = act.astype(np.int8)

    in_maps = []
    for c in range(NCORES):
        r0, r1 = c * BC, (c + 1) * BC
        actc = act8[r0:r1].reshape(GR, 128, A)[perm]
        rewc = rew[r0:r1].reshape(GR, 128)[perm]
        s1c = s1col[r0:r1].reshape(GR, 128)[perm]
        in_maps.append({
            "x0t": np.ascontiguousarray(st0[r0:r1].T).astype(NP_FP8),
            "x1t": np.ascontiguousarray(st1f[r0:r1].T).astype(NP_FP8),
            "actb": np.ascontiguousarray(
                actc.transpose(1, 0, 2).reshape(128, GR * A)),
            "rewb": np.ascontiguousarray(rewc.transpose(1, 0)),
            "s1b": np.ascontiguousarray(s1c.transpose(1, 0)),
            "w1": w1f, "w2km": w2km, "w3s": w3s,
            "b1d": b1m, "b2d": b2m, "b3st": b3stk, "iotad": iota,
        })
    return in_maps


def _run(inputs, trace=False):
    nc = _get_program()
    in_maps = _prep_in_maps(inputs)
    res = run_bass_kernel_spmd(nc, in_maps, core_ids=list(range(NCORES)),
                               trace=trace)
    total = 0.0
    for r in res.results:
        total += float(np.asarray(r["outp"], dtype=np.float64).sum())
    return np.array(np.float32(total)), res


def kernel(**inputs) -> np.ndarray:
    val, _ = _run(inputs, trace=False)
    return val


# revision 35
# speedup vs baseline: 1.2968x; 1.2968x over previous
"""Bass/Trainium2 kernel for nn_BellmanLoss (8-core data-parallel).

Math: the reference's scatter makes Q_new differ from Q0 only at
a_i = argmax_j(actions[i, j]) (first max), so

    loss = sum_i (Q0[i, a_i] - target_i)^2
    target_i = r_i + 0.9 * max_a Qn[i, a] * (1 - done_i),  done_i = (states1[i,0] == 666)

Per core: 8192 rows, CH=512 batch columns per tick, 32 ticks (even=state0,
odd=state1 chunks). MLP runs feature-major (h^T = [features, batch]):
  mm1: fp8 non-DR (K=128), N=512  -> h1p PSUM [128,2,512]
  relu1: ACT/DVE copy PSUM->SBUF fp8 (+b1)
  mm2: fp8 DoubleRow (K=256 packed), N=512 -> h2p PSUM [128,512] per m
  relu2: per-m copies -> h2s fp8
  mm3: fp8 DoubleRow, Q^T [18,512] packed 4x32-partition groups per qt bank
  stack: PSUM->SBUF bf16 (+b3)
  dma_start_transpose: qs [128,512] -> qbuf [128,4,128] batch-major
Epilogue (batch-major, small FD): argmax-onehot select of Q0, max of Qn,
target, per-partition loss partials. Host does layout-only prep and the
final 1024-element sum.
"""

import os
os.environ.setdefault("NEURON_RT_ENABLE_DGE_NOTIFICATIONS", "1")
import numpy as np
import ml_dtypes

import concourse.bass as bass
import concourse.mybir as mybir
import concourse.tile as tile
from concourse import bacc
from concourse.bass_utils import run_bass_kernel_spmd

# Problem constants (hardcoded per contract)
B, S, H, A = 65536, 128, 256, 18
NCORES = 8
BC = B // NCORES          # 8192 rows per core
CH = 512                  # batch columns per tick
T = 2 * (BC // CH)        # 32 ticks (x0/x1 interleaved)
NQ = BC // CH // 2        # 8 qt tiles (each: 2 chunk-pairs x (Q0,Qn))
GR = BC // 128            # 64 batch blocks of 128 rows
LOADCOLS = 1024           # x DMA tile columns
DONE = 666.0
DISC = 0.9

FP8 = mybir.dt.float8e4
BF16 = mybir.dt.bfloat16
FP16 = mybir.dt.float16
F32 = mybir.dt.float32
I8 = mybir.dt.int8
AF = mybir.ActivationFunctionType
OP = mybir.AluOpType
AX = mybir.AxisListType
DR = mybir.MatmulPerfMode.DoubleRow

NP_FP8 = ml_dtypes.float8_e4m3
NP_BF16 = ml_dtypes.bfloat16

USE_DR = os.environ.get("BELLMAN_DR", "1") == "1"
USE_DMAT = os.environ.get("BELLMAN_DMAT", "1") == "1"


def _build_program():
    nc = bacc.Bacc("TRN2", target_bir_lowering=False, debug=False)

    x0t = nc.dram_tensor("x0t", [128, BC], FP8, kind="ExternalInput").ap()
    x1t = nc.dram_tensor("x1t", [128, BC], FP8, kind="ExternalInput").ap()
    actb = nc.dram_tensor("actb", [128, GR * A], I8, kind="ExternalInput").ap()
    rewb = nc.dram_tensor("rewb", [128, GR], F32, kind="ExternalInput").ap()
    s1b = nc.dram_tensor("s1b", [128, GR], F32, kind="ExternalInput").ap()
    w1 = nc.dram_tensor("w1", [S, H], FP8, kind="ExternalInput").ap()
    w2km = nc.dram_tensor("w2km", [128, 2 * H], FP8, kind="ExternalInput").ap()
    w3s = nc.dram_tensor("w3s", [128, 2 * 32], FP8, kind="ExternalInput").ap()
    b1d = nc.dram_tensor("b1d", [128, 2], F32, kind="ExternalInput").ap()
    b2d = nc.dram_tensor("b2d", [128, 2], F32, kind="ExternalInput").ap()
    b3st = nc.dram_tensor("b3st", [128, 1], F32, kind="ExternalInput").ap()
    iotad = nc.dram_tensor("iotad", [128, A], FP16, kind="ExternalInput").ap()
    outp = nc.dram_tensor("outp", [128, 1], F32, kind="ExternalOutput").ap()

    from contextlib import ExitStack

    with tile.TileContext(nc) as tc, ExitStack() as ctx:
        singles = ctx.enter_context(tc.tile_pool(name="singles", bufs=1))
        xpool = ctx.enter_context(tc.tile_pool(name="xpool", bufs=3))
        h1spool = ctx.enter_context(tc.tile_pool(name="h1s", bufs=4))
        h2spool = ctx.enter_context(tc.tile_pool(name="h2s", bufs=6))
        big = ctx.enter_context(tc.tile_pool(name="big", bufs=1))
        ps_h1 = ctx.enter_context(tc.tile_pool(name="ps_h1", bufs=2, space="PSUM"))
        ps_h2 = ctx.enter_context(tc.tile_pool(name="ps_h2", bufs=3, space="PSUM"))
        ps_qt = ctx.enter_context(tc.tile_pool(name="ps_qt", bufs=1, space="PSUM"))

        # --- constants / per-core staging loads (scalar queue, early) ---
        w1_s = singles.tile([S, H], FP8)
        nc.scalar.dma_start(out=w1_s, in_=w1)
        w2_s = singles.tile([128, 2, H], FP8, tag="w2")
        nc.scalar.dma_start(
            out=w2_s[:, :, :].rearrange("p a b -> p (a b)"), in_=w2km)
        w3_s = singles.tile([128, 2, 32], FP8, tag="w3")
        nc.scalar.dma_start(
            out=w3_s[:, :, :].rearrange("p a b -> p (a b)"), in_=w3s)
        b1_s = singles.tile([128, 2], F32, tag="b1")
        nc.scalar.dma_start(out=b1_s, in_=b1d)
        b2_s = singles.tile([128, 2], F32, tag="b2")
        nc.scalar.dma_start(out=b2_s, in_=b2d)
        b3_s = singles.tile([128, 1], F32, tag="b3")
        nc.scalar.dma_start(out=b3_s, in_=b3st)
        iota_s = singles.tile([128, A], FP16, tag="iota")
        nc.scalar.dma_start(out=iota_s, in_=iotad)
        actb_s = singles.tile([128, GR * A], I8, tag="actb")
        rewb_s = singles.tile([128, GR], F32, tag="rewb")
        s1b_s = singles.tile([128, GR], F32, tag="s1b")

        # qs: stacked Q^T in SBUF bf16 (stack copies write, dma-transpose reads)
        qs = big.tile([128, NQ, CH], BF16, tag="qs")
        # qbuf: batch-major Q (partition = batch-within-128-block)
        qbuf = big.tile([128, NQ, 4, 128], BF16, tag="qbuf")

        # epilogue tiles
        actf = big.tile([128, GR * A], FP16, tag="actf")
        score = big.tile([128, GR, A], FP16, tag="score")
        rowmax = big.tile([128, GR], FP16, tag="rowmax")
        onehot = big.tile([128, GR, A], BF16, tag="onehot")
        donem = big.tile([128, GR], F32, tag="donem")
        fac = big.tile([128, GR], F32, tag="fac")
        prod = big.tile([128, GR, A], BF16, tag="prod")
        q0sel = big.tile([128, GR], F32, tag="q0sel")
        maxqn = big.tile([128, GR], F32, tag="maxqn")
        t1 = big.tile([128, GR], F32, tag="t1")
        t2 = big.tile([128, GR], F32, tag="t2")
        diff = big.tile([128, GR], F32, tag="diff")
        sq = big.tile([128, GR], F32, tag="sq")
        acc = big.tile([128, 1], F32, tag="acc")

        xL = {}
        h1p_t, h1s_t, h2pa_t, h2pb_t, h2s_t, qt_q = {}, {}, {}, {}, {}, {}

        # greedy engine balancer for PSUM-evacuation copies
        eng_load = {"a": 0.0, "v": 0.0}

        def pick_engine(act_cost, dve_cost):
            if eng_load["a"] + act_cost <= eng_load["v"] + dve_cost:
                eng_load["a"] += act_cost
                return nc.scalar
            eng_load["v"] += dve_cost
            return nc.vector

        def do_dma(li):
            x0L = xpool.tile([128, LOADCOLS], FP8, tag="x0")
            x1L = xpool.tile([128, LOADCOLS], FP8, tag="x1")
            nc.sync.dma_start(out=x0L,
                              in_=x0t[:, li * LOADCOLS:(li + 1) * LOADCOLS])
            nc.sync.dma_start(out=x1L,
                              in_=x1t[:, li * LOADCOLS:(li + 1) * LOADCOLS])
            xL[li] = (x0L, x1L)

        # small first loads so tick 0/1 start ~2us earlier
        x0m = singles.tile([128, CH], FP8, tag="x0m")
        nc.sync.dma_start(out=x0m, in_=x0t[:, 0:CH])
        x1m = singles.tile([128, CH], FP8, tag="x1m")
        nc.sync.dma_start(out=x1m, in_=x1t[:, 0:CH])

        def xs_for(t):
            if t == 0:
                return x0m[:, :]
            if t == 1:
                return x1m[:, :]
            c, pa = t // 2, t % 2
            li = (c * CH) // LOADCOLS
            ci = (c * CH) % LOADCOLS // CH
            return xL[li][pa][:, ci * CH:(ci + 1) * CH]

        def st_mm1(t):
            h1p = ps_h1.tile([128, 2, CH], F32, tag="h1p", name=f"h1p_{t}")
            xs = xs_for(t)
            for m in range(2):
                nc.tensor.matmul(h1p[:, m, :], w1_s[:, m * 128:(m + 1) * 128],
                                 xs, start=True, stop=True)
            h1p_t[t] = h1p

        def emit_relu(eng, dst, src, bias_ap):
            if eng is nc.scalar:
                nc.scalar.activation(dst, src, AF.Relu, bias=bias_ap, scale=1.0)
            else:
                nc.vector.tensor_scalar(dst, src, bias_ap, 0.0, OP.add, OP.max)

        def st_relu1(t):
            h1s = h1spool.tile([128, 2, CH], FP8, tag="h1s", name=f"h1s_{t}")
            emit_relu(pick_engine(1114, 1282),
                      h1s[:, :, :].rearrange("p a b -> p (a b)"),
                      h1p_t.pop(t)[:, :, :].rearrange("p a b -> p (a b)"),
                      b1_s[:, 0:1])
            h1s_t[t] = h1s

        def st_mm2(t):
            h1s = h1s_t.pop(t)
            for m, store in ((0, h2pa_t), (1, h2pb_t)):
                h2p = ps_h2.tile([128, CH], F32, tag="h2p",
                                 name=f"h2p{m}_{t}")
                if USE_DR:
                    nc.tensor.matmul(h2p, w2_s[:, :, m * 128:(m + 1) * 128],
                                     h1s[:, :, :], start=True, stop=True,
                                     perf_mode=DR)
                else:
                    for k in range(2):
                        nc.tensor.matmul(h2p,
                                         w2_s[:, k, m * 128:(m + 1) * 128],
                                         h1s[:, k, :],
                                         start=(k == 0), stop=(k == 1))
                store[t] = h2p

        def st_relu2(t):
            h2s = h2spool.tile([128, 2, CH], FP8, tag="h2s", name=f"h2s_{t}")
            emit_relu(pick_engine(687, 742), h2s[:, 0, :], h2pa_t.pop(t),
                      b2_s[:, 0:1])
            emit_relu(pick_engine(687, 742), h2s[:, 1, :], h2pb_t.pop(t),
                      b2_s[:, 1:2])
            h2s_t[t] = h2s

        def st_mm3(c):
            # chunk-pair c: Q0 from h2s[2c] (state0), Qn from h2s[2c+1];
            # col-tiled pairs run concurrently on different PE column groups
            q = c // 2
            gp = (c % 2) * 2
            if c % 2 == 0:
                qt_q[q] = ps_qt.tile([128, CH], F32, tag="qt", name=f"qt_{q}")
            qt = qt_q[q]
            h2s0 = h2s_t.pop(2 * c)
            h2s1 = h2s_t.pop(2 * c + 1)
            for k in range(2):
                for gi, h2sx in ((gp, h2s0), (gp + 1, h2s1)):
                    po = gi * 32
                    nc.tensor.matmul(qt[po:po + A, :], w3_s[:, k, 0:A],
                                     h2sx[:, k, :], start=(k == 0),
                                     stop=(k == 1), tile_position=(0, po))

        def st_stack(q):
            # PSUM f32 -> SBUF bf16 with b3 bias (per stacked partition)
            eng = pick_engine(690, 750)
            if eng is nc.scalar:
                nc.scalar.activation(qs[:, q, :], qt_q[q], AF.Identity,
                                     bias=b3_s[:, 0:1], scale=1.0)
            else:
                nc.vector.tensor_scalar(qs[:, q, :], qt_q[q], b3_s[:, 0:1],
                                        None, OP.add)
            qt_q.pop(q)

        def st_dmaT(q):
            nc.sync.dma_start_transpose(out=qbuf[:, q, :, :], in_=qs[:, q, :])

        def ep_front(qq):
            # quarter qq of argmax/onehot of actions; done mask; factor
            gsl = slice(qq * 16, (qq + 1) * 16)
            asl = slice(qq * 16 * A, (qq + 1) * 16 * A)
            a3 = actf[:, asl].rearrange("p (g a) -> p g a", a=A)
            iot_b = iota_s[:, None, :].broadcast_to([128, 16, A])
            nc.scalar.activation(actf[:, asl], actb_s[:, asl], AF.Copy,
                                 scale=32.0)
            eng_load["a"] += 530
            nc.vector.tensor_tensor(score[:, gsl, :], a3, iot_b, OP.subtract)
            nc.vector.tensor_reduce(rowmax[:, gsl], score[:, gsl, :], AX.X,
                                    OP.max)
            nc.vector.tensor_tensor(
                onehot[:, gsl, :], score[:, gsl, :],
                rowmax[:, gsl, None].broadcast_to([128, 16, A]), OP.is_equal)
            eng_load["v"] += 1100
            if qq == 0:
                nc.vector.tensor_scalar(donem, s1b_s, DONE, None, OP.is_equal)
                nc.vector.tensor_scalar(fac, donem, -DISC, DISC, OP.mult,
                                        OP.add)
                eng_load["v"] += 370

        def ep_tail(half):
            # half = 0..3: q pair (2*half, 2*half+1), blocks 16*half..16*half+15
            q0_ = half * 2
            qh = qbuf[:, q0_:q0_ + 2, :, :]          # [128, 2, 4, 128]
            gsl = slice(half * 16, (half + 1) * 16)
            oh3 = onehot[:, :, :]                     # [128, GR, A]
            for pair in range(2):
                # blocks g' = 8q + 2w + pair; qbuf group = 2*pair (+1 Qn)
                q0ap = qh[:, :, :, 64 * pair:64 * pair + A]       # [128,2,4,A]
                qnap = qh[:, :, :, 64 * pair + 32:64 * pair + 32 + A]
                ohap = oh3[:, half * 16 + pair:(half + 1) * 16:2, :] \
                    .rearrange("p (q w) a -> p q w a", q=2)
                prap = prod[:, half * 16 + pair:(half + 1) * 16:2, :] \
                    .rearrange("p (q w) a -> p q w a", q=2)
                nc.vector.tensor_tensor(prap, ohap, q0ap, OP.mult)
                nc.vector.tensor_reduce(
                    q0sel[:, half * 16 + pair:(half + 1) * 16:2]
                    .rearrange("p (q w) -> p q w", q=2), prap, AX.X, OP.add)
                nc.vector.tensor_reduce(
                    maxqn[:, half * 16 + pair:(half + 1) * 16:2]
                    .rearrange("p (q w) -> p q w", q=2), qnap, AX.X, OP.max)
            nc.vector.tensor_tensor(t1[:, gsl], maxqn[:, gsl], fac[:, gsl],
                                    OP.mult)
            nc.vector.tensor_tensor(t2[:, gsl], t1[:, gsl], rewb_s[:, gsl],
                                    OP.add)
            nc.vector.tensor_tensor(diff[:, gsl], q0sel[:, gsl], t2[:, gsl],
                                    OP.subtract)
            nc.vector.tensor_tensor(sq[:, gsl], diff[:, gsl], diff[:, gsl],
                                    OP.mult)
            eng_load["v"] += 2400

        # ---- main software-pipelined loop ----
        # Prime the PE HAM clock-gate with ~3.5us of throwaway matmuls on
        # uninitialized SBUF (no DMA deps, so they start right after the
        # preamble) so the array is at 2.4GHz when the real work lands.
        prime_ps = ps_qt.tile([128, CH], F32, tag="qt", name="prime")
        for _ in range(9):
            nc.tensor.matmul(prime_ps, qs[:, 0, 0:128], qs[:, 0, :],
                             start=True, stop=True)
        do_dma(0)
        do_dma(1)
        PASS_PER_LOAD = 2 * LOADCOLS // CH   # ticks covered per load pair
        for t in range(T + 10):
            nt = t + 2 * PASS_PER_LOAD
            if nt < T and nt % PASS_PER_LOAD == 0:
                do_dma(nt // PASS_PER_LOAD)
            if t == 3:
                nc.sync.dma_start(out=actb_s, in_=actb)
                nc.sync.dma_start(out=rewb_s, in_=rewb)
                nc.sync.dma_start(out=s1b_s, in_=s1b)
            if t in (7, 9, 11, 13):
                ep_front((t - 7) // 2)
            if t < T:
                st_mm1(t)
            if 0 <= t - 2 < T:
                st_mm2(t - 2)
            if t >= 5 and (t - 5) % 2 == 0 and (t - 5) // 2 < T // 2:
                st_mm3((t - 5) // 2)
            if 0 <= t - 3 < T:
                st_relu2(t - 3)
            if 0 <= t - 1 < T:
                st_relu1(t - 1)
            if t >= 8 and (t - 8) % 4 == 0:
                q = (t - 8) // 4
                if q < NQ:
                    st_stack(q)
            if t >= 9 and (t - 9) % 4 == 0:
                q = (t - 9) // 4
                if q < NQ:
                    st_dmaT(q)
            if t >= 15 and (t - 15) % 8 == 0 and (t - 15) // 8 < 4:
                ep_tail((t - 15) // 8)
        nc.vector.tensor_reduce(acc, sq, AX.X, OP.add)
        nc.scalar.dma_start(out=outp, in_=acc)

    nc.compile()
    return nc


_CACHE = {}


def _get_program():
    if "nc" not in _CACHE:
        _CACHE["nc"] = _build_program()
    return _CACHE["nc"]


def _block_perm():
    # qbuf block order g' -> source batch block b
    perm = np.empty(GR, np.int64)
    for gp in range(GR):
        q, r = divmod(gp, 8)
        w, pair = divmod(r, 2)
        perm[gp] = (2 * q + pair) * 4 + w
    return perm


def _prep_in_maps(inputs):
    st0 = np.asarray(inputs["states0"], dtype=np.float32)
    st1 = np.asarray(inputs["states1"], dtype=np.float32)
    act = np.asarray(inputs["actions"], dtype=np.int32)
    rew = np.asarray(inputs["rewards"], dtype=np.float32)
    W1 = np.asarray(inputs["W1"], dtype=np.float32)
    W2 = np.asarray(inputs["W2"], dtype=np.float32)
    W3 = np.asarray(inputs["W3"], dtype=np.float32)
    b1 = np.asarray(inputs["b1"], dtype=np.float32)
    b2 = np.asarray(inputs["b2"], dtype=np.float32)
    b3 = np.asarray(inputs["b3"], dtype=np.float32)

    # sanitize DONE sentinel (666 > fp8e4m3 max); done rows' Qn is masked out
    s1col = st1[:, 0].copy()
    st1f = st1.copy()
    st1f[:, 0] = np.where(s1col == DONE, 0.0, s1col)

    w1f = W1.astype(NP_FP8)
    w2km = np.ascontiguousarray(
        W2.reshape(2, 128, H).transpose(1, 0, 2)).astype(NP_FP8).reshape(128, 2 * H)
    w3p = np.zeros((128, 2, 32), np.float32)
    w3p[:, :, :A] = W3.reshape(2, 128, A).transpose(1, 0, 2)
    w3s = w3p.astype(NP_FP8).reshape(128, 2 * 32)
    b1m = np.ascontiguousarray(b1.reshape(2, 128).T)
    b2m = np.ascontiguousarray(b2.reshape(2, 128).T)
    b3stk = np.zeros((128, 1), np.float32)
    for g in range(4):
        b3stk[g * 32:g * 32 + A, 0] = b3
    iota = np.ascontiguousarray(
        np.broadcast_to(np.arange(A, dtype=np.float16), (128, A)))

    perm = _block_perm()
    act8 = act.astype(np.int8)

    in_maps = []
    for c in range(NCORES):
        r0, r1 = c * BC, (c + 1) * BC
        actc = act8[r0:r1].reshape(GR, 128, A)[perm]
        rewc = rew[r0:r1].reshape(GR, 128)[perm]
        s1c = s1col[r0:r1].reshape(GR, 128)[perm]
        in_maps.append({
            "x0t": np.ascontiguousarray(st0[r0:r1].T).astype(NP_FP8),
            "x1t": np.ascontiguousarray(st1f[r0:r1].T).astype(NP_FP8),
            "actb": np.ascontiguousarray(
                actc.transpose(1, 0, 2).reshape(128, GR * A)),
            "rewb": np.ascontiguousarray(rewc.transpose(1, 0)),
            "s1b": np.ascontiguousarray(s1c.transpose(1, 0)),
            "w1": w1f, "w2km": w2km, "w3s": w3s,
            "b1d": b1m, "b2d": b2m, "b3st": b3stk, "iotad": iota,
        })
    return in_maps


def _run(inputs, trace=False):
    nc = _get_program()
    in_maps = _prep_in_maps(inputs)
    res = run_bass_kernel_spmd(nc, in_maps, core_ids=list(range(NCORES)),
                               trace=trace)
    total = 0.0
    for r in res.results:
        total += float(np.asarray(r["outp"], dtype=np.float64).sum())
    return np.array(np.float32(total)), res


def kernel(**inputs) -> np.ndarray:
    val, _ = _run(inputs, trace=False)
    return val
